# revision 13
# baseline (speedup 1.0000x reference)
"""Trainium2 Bass kernel for nn_CustomMoEBranch (STFT-gated MoE, top-2 of 8 experts).

Strategy (8 NeuronCores, expert-parallel):
  - Core e receives expert e's weight slice [4096, 1024] plus the full x and
    gating weights. Each core computes the full frequency-domain gating
    (windowed 256-pt rDFT as matmuls -> |.| -> time-mean -> 2-layer MLP ->
    exact top-2 softmax) and its own expert GEMM, scales the expert output by
    the per-sample routing weight for expert e (0 when not selected), and
    writes a [256, 1024] partial. The host sums the 8 partials - that equals
    the reference's gather+weighted-combine because non-selected experts get
    weight exactly 0.
  - All matmuls run in f32r (fp32 with 11-bit mantissa, full PE rate). The
    routing decision needs ~1e-4 relative precision (top-2 margins on logits);
    f32r delivers ~1.5e-4 per matmul which keeps the selection exact and the
    final absmax relative error ~1e-3.
  - Host-side prep is layout/dtype only: reflect-pad, the [B, 34*128] ->
    [128, B*34] block transpose (so the contraction dims of both the DFT and
    the expert GEMM sit on SBUF partitions), and f32r rounding.

Self-contained: hardcodes all shapes; no file reads.
"""

import numpy as np

import concourse.bass as bass
import concourse.mybir as mybir
import concourse.tile as tile
from concourse import bacc
from concourse.bass_utils import run_bass_kernel_spmd
from concourse.masks import make_identity

dt = mybir.dt
AF = mybir.ActivationFunctionType
ALU = mybir.AluOpType
AX = mybir.AxisListType

B = 256          # batch
L = 4096         # signal length
E = 8            # experts
D = 1024         # expert output dim
H = 128          # gate hidden
NFFT = 256
HOP = 128
F = 129          # rfft bins
T = 33           # stft frames
I = 34           # 128-wide blocks in padded signal (4352 / 128)
P = 128          # partitions
N_CORES = 8

BGRP = 8                     # samples per DFT matmul group
NGRP = B // BGRP             # 32 groups
GCOLS = BGRP * I             # 272 moving rows per DFT matmul (34th col junk)
KL = L // P                  # 32 contraction chunks for the expert GEMM
ND = D // 512                # 2 psum-width chunks of D
MB = B // P                  # 2 batch chunks


def _round_f32r(x: np.ndarray) -> np.ndarray:
    """Round fp32 to f32r (11-bit mantissa, RNE). Same bit layout as fp32."""
    b = np.ascontiguousarray(x, dtype=np.float32).view(np.uint32)
    r = (b.astype(np.uint64) + 0x7FF + ((b >> 12) & 1)) & 0xFFFFF000
    return r.astype(np.uint32).view(np.float32)


def _dft_consts():
    """Windowed rDFT matrices, f32r-rounded, packed per 128-row chunk.

    A covers cos f=0..127; Bs covers sin f=0..127 (the f=0 column is all
    zeros, so |X_f|^2 = A[f]^2 + Bs[f]^2 holds partition-aligned for
    f=0..127). The f=128 (Nyquist) bin is a separate 1-column stationary
    (windowed alternating-sign cosine); its magnitude is just |.|.
    Returns dftw [128, 514] = [A0 | Bs0 | A1 | Bs1 | c128_0 | c128_1]
    (chunk c covers sample index n = 128c + p).
    """
    win = 0.5 * (1.0 - np.cos(2.0 * np.pi * np.arange(NFFT) / NFFT))
    n = np.arange(NFFT)
    f = np.arange(F)
    ang = 2.0 * np.pi * np.outer(n, f) / NFFT
    C = (win[:, None] * np.cos(ang)).astype(np.float32)
    S = (win[:, None] * np.sin(ang)).astype(np.float32)
    out = np.empty((P, 4 * P), np.float32)
    for c in range(2):
        Cc = C[c * P:(c + 1) * P]
        Sc = S[c * P:(c + 1) * P]
        out[:, c * 256:c * 256 + 128] = Cc[:, :128]
        # B chunk: col 0 = Nyquist cosine (sin f=0 is all-zero anyway),
        # cols 1..127 = sin f=1..127. Row 0 of the B matmul output is the
        # Nyquist bin; rows 1..127 pair partition-aligned with A rows.
        out[:, c * 256 + 128] = Cc[:, 128]
        out[:, c * 256 + 129:c * 256 + 256] = Sc[:, 1:128]
    return _round_f32r(out)


_NC_CACHE = {}


def _build():
    if "nc" in _NC_CACHE:
        return _NC_CACHE["nc"]
    nc = bacc.Bacc("TRN2", target_bir_lowering=False, name="moe_branch")

    xbt = nc.dram_tensor("xbt", [P, B * I + I], dt.float32r, kind="ExternalInput")
    dftw = nc.dram_tensor("dftw", [P, 512], dt.float32r, kind="ExternalInput")
    gw = nc.dram_tensor("gw", [P, H + E], dt.float32r, kind="ExternalInput")
    gw1r = nc.dram_tensor("gw1r", [1, H], dt.float32r, kind="ExternalInput")
    gb = nc.dram_tensor("gb", [P, 2], dt.float32, kind="ExternalInput")
    esel = nc.dram_tensor("esel", [1, E], dt.float32, kind="ExternalInput")
    onesr = nc.dram_tensor("onesr", [1, P], dt.float32r, kind="ExternalInput")
    wexp = nc.dram_tensor("wexp", [L, D], dt.float32r, kind="ExternalInput")
    bexp = nc.dram_tensor("bexp", [1, D], dt.float32r, kind="ExternalInput")
    out = nc.dram_tensor("out", [B, D], dt.float32, kind="ExternalOutput")

    with tile.TileContext(nc) as tc:
        with (
            tc.tile_pool(name="const", bufs=1) as cp,
            tc.tile_pool(name="small", bufs=2) as sp,
            tc.tile_pool(name="mag", bufs=3) as mp,
            tc.tile_pool(name="wpool", bufs=6) as wp,
            tc.tile_pool(name="opool", bufs=2) as op,
            tc.tile_pool(name="psd", bufs=2, space="PSUM") as psd,
            tc.tile_pool(name="pse", bufs=1, space="PSUM") as pse,
        ):
            # ---- resident inputs
            X = cp.tile([P, B * I + I], dt.float32r, tag="X")
            for j in range(4):
                sl = slice(j * 2176, min(j * 2176 + 2176, B * I + I))
                nc.sync.dma_start(X[:, sl], xbt[:, sl])
            Xv = X[:, 0:B * I].rearrange("p (b i) -> p b i", i=I)

            dw = cp.tile([P, 512], dt.float32r, tag="dw")
            nc.sync.dma_start(dw[:], dftw[:])
            gwt = cp.tile([P, H + E], dt.float32r, tag="gwt")
            nc.sync.dma_start(gwt[:], gw[:])
            gw1rt = cp.tile([1, H], dt.float32r, tag="gw1rt")
            nc.sync.dma_start(gw1rt[:], gw1r[:])
            gbt = cp.tile([P, 2], dt.float32, tag="gbt")
            nc.sync.dma_start(gbt[:], gb[:])
            eselb = cp.tile([P, E], dt.float32, tag="eselb")
            nc.sync.dma_start(eselb[:], esel[:].to_broadcast((P, E)))
            bexpt = cp.tile([1, D], dt.float32r, tag="bexpt")
            nc.sync.dma_start(bexpt[:], bexp[:])
            ones1 = cp.tile([1, P], dt.float32r, tag="ones1")
            nc.sync.dma_start(ones1[:], onesr[:])
            ident8 = cp.tile([8, 8], dt.float32, tag="ident8")
            make_identity(nc, ident8[:])

            pooled = cp.tile([P, B], dt.float32, tag="pooled")
            plast = cp.tile([1, B], dt.float32, tag="plast")

            # ---- gating DFT + magnitude + time-mean
            for q in range(NGRP):
                psA = psd.tile([P, GCOLS], dt.float32, tag="psA")
                psB = psd.tile([P, GCOLS], dt.float32, tag="psB")
                mv0 = X[:, q * GCOLS:(q + 1) * GCOLS]
                mv1 = X[:, q * GCOLS + 1:(q + 1) * GCOLS + 1]
                nc.tensor.matmul(psA[:], dw[:, 0:128], mv0, start=True, stop=False)
                nc.tensor.matmul(psA[:], dw[:, 256:384], mv1, start=False, stop=True)
                nc.tensor.matmul(psB[:], dw[:, 128:256], mv0, start=True, stop=False)
                nc.tensor.matmul(psB[:], dw[:, 384:512], mv1, start=False, stop=True)

                sA = mp.tile([P, GCOLS], dt.float32, tag="sA")
                sB = mp.tile([P, GCOLS], dt.float32, tag="sB")
                nc.scalar.activation(sA[:], psA[:], AF.Square)
                nc.scalar.activation(sB[:], psB[:], AF.Square)
                m2 = mp.tile([P, GCOLS], dt.float32, tag="m2")
                nc.vector.tensor_tensor(m2[:], sA[:], sB[:], op=ALU.add)
                magm = mp.tile([P, GCOLS], dt.float32, tag="magm")
                nc.scalar.activation(magm[:], m2[:], AF.Sqrt)
                # row 0 of magm mixes f=0 with Nyquist -> fixed up below
                mag0 = mp.tile([1, GCOLS], dt.float32, tag="mag0")
                nc.scalar.activation(mag0[:], psA[0:1, :], AF.Abs)
                magl = mp.tile([1, GCOLS], dt.float32, tag="magl")
                nc.scalar.activation(magl[:], psB[0:1, :], AF.Abs)

                nc.vector.tensor_reduce(
                    pooled[:, q * BGRP:(q + 1) * BGRP],
                    magm[:].rearrange("p (b i) -> p b i", i=I)[:, :, 0:T],
                    axis=AX.X,
                    op=ALU.add,
                )
                nc.vector.tensor_reduce(
                    pooled[0:1, q * BGRP:(q + 1) * BGRP],
                    mag0[:].rearrange("p (b i) -> p b i", i=I)[:, :, 0:T],
                    axis=AX.X,
                    op=ALU.add,
                )
                nc.vector.tensor_reduce(
                    plast[:, q * BGRP:(q + 1) * BGRP],
                    magl[:].rearrange("p (b i) -> p b i", i=I)[:, :, 0:T],
                    axis=AX.X,
                    op=ALU.add,
                )

            # (1/33 mean folded into gw on the host)
            pooled_r = cp.tile([P, B], dt.float32r, tag="pooled_r")
            nc.vector.tensor_copy(pooled_r[:], pooled[:])
            plast_r = cp.tile([1, B], dt.float32r, tag="plast_r")
            nc.vector.tensor_copy(plast_r[:], plast[:])

            # ---- gate MLP: hT = relu(g1.T @ pooled + b1); logT = g2.T @ hT + b2
            psH = psd.tile([P, B], dt.float32, tag="psA")
            nc.tensor.matmul(psH[:], gwt[:, 0:H], pooled_r[:], start=True, stop=False)
            nc.tensor.matmul(psH[:], gw1rt[:], plast_r[:], start=False, stop=True)
            hT = cp.tile([P, B], dt.float32r, tag="hT")
            nc.scalar.activation(hT[:], psH[:], AF.Relu, bias=gbt[:, 0:1])

            psL = psd.tile([8, B], dt.float32, tag="psB")
            nc.tensor.matmul(psL[:], gwt[:, H:H + E], hT[:], start=True, stop=True)
            logT = cp.tile([8, B], dt.float32, tag="logT")
            nc.scalar.activation(logT[:], psL[:], AF.Identity, bias=gbt[0:8, 1:2])

            # ---- top-2 softmax -> per-sample weight for this core's expert
            wcols = []
            for mb in range(MB):
                psT = psd.tile([P, 8], dt.float32, tag="psB")
                nc.tensor.transpose(
                    psT[:], logT[:, mb * P:(mb + 1) * P], ident8[:]
                )
                lg = sp.tile([P, 8], dt.float32, tag="lg")
                nc.vector.tensor_copy(lg[:], psT[:])

                m1v = sp.tile([P, 1], dt.float32, tag="m1v")
                nc.vector.tensor_reduce(m1v[:], lg[:], axis=AX.X, op=ALU.max)
                negm = sp.tile([P, 1], dt.float32, tag="negm")
                nc.vector.tensor_scalar_mul(negm[:], m1v[:], -1.0)
                mask1 = sp.tile([P, 8], dt.float32, tag="mask1")
                nc.vector.tensor_scalar(
                    mask1[:], lg[:], m1v[:, 0:1], None, op0=ALU.is_equal
                )
                big = sp.tile([P, 8], dt.float32, tag="big")
                nc.vector.tensor_scalar_mul(big[:], mask1[:], 1.0e30)
                lm = sp.tile([P, 8], dt.float32, tag="lm")
                nc.vector.tensor_tensor(lm[:], lg[:], big[:], op=ALU.subtract)
                m2v = sp.tile([P, 1], dt.float32, tag="m2v")
                nc.vector.tensor_reduce(m2v[:], lm[:], axis=AX.X, op=ALU.max)

                ex = sp.tile([P, 8], dt.float32, tag="ex")
                nc.scalar.activation(ex[:], lg[:], AF.Exp, bias=negm[:, 0:1])
                mask2 = sp.tile([P, 8], dt.float32, tag="mask2")
                nc.vector.tensor_scalar(
                    mask2[:], lg[:], m2v[:, 0:1], None, op0=ALU.is_ge
                )
                exm = sp.tile([P, 8], dt.float32, tag="exm")
                nc.vector.tensor_tensor(exm[:], ex[:], mask2[:], op=ALU.mult)
                zz = sp.tile([P, 1], dt.float32, tag="zz")
                nc.vector.tensor_reduce(zz[:], exm[:], axis=AX.X, op=ALU.add)
                rz = sp.tile([P, 1], dt.float32, tag="rz")
                nc.vector.reciprocal(rz[:], zz[:])
                wall = sp.tile([P, 8], dt.float32, tag="wall")
                nc.vector.tensor_scalar_mul(wall[:], exm[:], rz[:, 0:1])
                wsel = sp.tile([P, 8], dt.float32, tag="wsel")
                nc.vector.tensor_tensor(wsel[:], wall[:], eselb[:], op=ALU.mult)
                wcol = cp.tile([P, 1], dt.float32, tag=f"wcol{mb}")
                nc.vector.tensor_reduce(wcol[:], wsel[:], axis=AX.X, op=ALU.add)
                wcols.append(wcol)

            # ---- expert GEMM: out[b, d] = wcol[b] * (x[b] @ W + bias)
            psE = []
            for mb in range(MB):
                row = []
                for nd in range(ND):
                    pet = pse.tile([P, 512], dt.float32, tag=f"e{mb}{nd}")
                    row.append(pet)
                psE.append(row)
            for k in range(KL):
                wk = wp.tile([P, D], dt.float32r, tag="wk")
                nc.sync.dma_start(wk[:], wexp[k * P:(k + 1) * P, :])
                for nd in range(ND):
                    for mb in range(MB):
                        nc.tensor.matmul(
                            psE[mb][nd][:],
                            Xv[:, mb * P:(mb + 1) * P, k + 1],
                            wk[:, nd * 512:(nd + 1) * 512],
                            start=(k == 0),
                            stop=False,
                        )
            for nd in range(ND):
                for mb in range(MB):
                    nc.tensor.matmul(
                        psE[mb][nd][:],
                        ones1[:],
                        bexpt[:, nd * 512:(nd + 1) * 512],
                        start=False,
                        stop=True,
                    )
                    ot = op.tile([P, 512], dt.float32, tag="ot")
                    nc.scalar.activation(
                        ot[:], psE[mb][nd][:], AF.Copy, scale=wcols[mb][:, 0:1]
                    )
                    nc.sync.dma_start(
                        out[mb * P:(mb + 1) * P, nd * 512:(nd + 1) * 512], ot[:]
                    )

    nc.compile()
    _NC_CACHE["nc"] = nc
    return nc


def _host_inputs(x, gate_w1, gate_b1, gate_w2, gate_b2, expert_w, expert_b):
    xp = np.pad(x.astype(np.float32), ((0, 0), (HOP, HOP)), mode="reflect")
    xbt = _round_f32r(np.ascontiguousarray(xp.reshape(B, I, P).transpose(2, 0, 1)))
    xbt = np.concatenate(
        [xbt.reshape(P, B * I), np.zeros((P, I), np.float32)], axis=1
    )

    dftw = _dft_consts()
    g1 = gate_w1.astype(np.float32) / T  # fold the time-mean into the MLP
    gw_pack = np.zeros((P, H + E), np.float32)
    gw_pack[:, 0:H] = g1[0:P, :]
    gw_pack[0:H, H:H + E] = gate_w2.astype(np.float32)
    gw_pack = _round_f32r(gw_pack)
    gw1r = _round_f32r(g1[P:P + 1, :])
    gb_pack = np.zeros((P, 2), np.float32)
    gb_pack[:, 0] = gate_b1.astype(np.float32)
    gb_pack[0:E, 1] = gate_b2.astype(np.float32)

    in_maps = []
    for e in range(N_CORES):
        sel = np.zeros((1, E), np.float32)
        sel[0, e] = 1.0
        in_maps.append(
            {
                "xbt": xbt,
                "dftw": dftw,
                "gw": gw_pack,
                "gw1r": gw1r,
                "gb": gb_pack,
                "onesr": np.ones((1, P), np.float32),
                "esel": sel,
                "wexp": _round_f32r(expert_w[e].astype(np.float32)),
                "bexp": _round_f32r(expert_b[e].astype(np.float32)[None, :]),
            }
        )
    return in_maps


def kernel(x, gate_w1, gate_b1, gate_w2, gate_b2, expert_w, expert_b):
    nc = _build()
    in_maps = _host_inputs(
        x, gate_w1, gate_b1, gate_w2, gate_b2, expert_w, expert_b
    )
    res = run_bass_kernel_spmd(nc, in_maps, core_ids=list(range(N_CORES)))
    acc = np.zeros((B, D), np.float64)
    for r in res.results:
        acc += r["out"].astype(np.float64)
    return acc.astype(np.float32)
